# revision 1
# baseline (speedup 1.0000x reference)
"""Two-layer GAT (PyG GATConv semantics) on 8 Trainium2 NeuronCores.

Strategy (dst-sharded, edge chunks of 128 on partitions):
- Shard destination nodes contiguously across the 8 cores (6250 each).
- Host preprocessing (integer graph data only): add self-loops, sort edges
  by dst, split per core, group per 128-dst tile, pad each tile's edge runs
  to multiples of 128, and build per-chunk selection matrices
  (Sel [128e x 128j] / SelT [128j x 128e]) plus src-index lists.
- Device, per layer: project node features (h = x @ W plus folded attention
  logit columns and skip projection), AllGather the per-node table rows
  [h(128) | a_s(2)], then per dst tile: indirect-DMA gather of src rows,
  per-edge attention weights w = max(exp(z), exp(0.2 z)) with z = a_s + a_d
  (exp(leaky_relu) factorization; softmax scale-invariance makes the max
  subtraction unnecessary), weighted aggregation + denominators via PE
  matmuls accumulating in PSUM, then normalize, add skip, relu.
"""

import sys

if "/opt/trn_rl_repo" not in sys.path:
    sys.path.insert(0, "/opt/trn_rl_repo")

import numpy as np

import concourse.bass as bass
import concourse.mybir as mybir
import concourse.tile as tile
from concourse.bass_utils import run_bass_kernel_spmd
from concourse.masks import make_identity

N, E, F_IN, H, C = 50000, 800000, 128, 2, 64
HC = H * C
NCORES = 8
SHARD = N // NCORES            # 6250
P = 128
TILES = (SHARD + P - 1) // P   # 49
NPAD = TILES * P               # 6272
ROW = 132                      # table row: h(128) | a_s(2) | pad(2)
PRJ = 260                      # proj cols: W(128) | w_as(2) | w_ad(2) | Wsk(128)

F32 = mybir.dt.float32
BF16 = mybir.dt.bfloat16
I32 = mybir.dt.int32
NP_BF16 = mybir.dt.np(BF16)


def _split_sync_waits(nc, limit=1):
    """walrus in this container rejects >1 sync wait per instruction; move
    excess waits onto NoOps inserted just before the offending one."""
    ctr = [0]

    def fresh_noop(engine, waits):
        ctr[0] += 1
        return mybir.InstNoOp(
            name=f"waitsplit-{ctr[0]}",
            engine=engine,
            bass_nofuse=True,
            sync_info=mybir.SyncInfo(on_wait=list(waits), on_update=[]),
        )

    for f in nc.m.functions:
        for bb in f.blocks:
            out = []
            changed = False
            for ins in bb.instructions:
                si = ins.sync_info
                waits = list(si.on_wait) if si else []
                if len(waits) > limit:
                    changed = True
                    excess, keep = waits[:-limit], waits[-limit:]
                    for i in range(0, len(excess), limit):
                        noop = fresh_noop(ins.engine, excess[i : i + limit])
                        nc.register_instruction(noop, overwrite=True)
                        out.append(noop)
                    ins.sync_info = mybir.SyncInfo(
                        on_wait=keep, on_update=list(si.on_update)
                    )
                out.append(ins)
            if changed:
                bb.instructions = out
    return ctr[0]


def _host_prep(src, dst):
    s = np.concatenate([src.astype(np.int64), np.arange(N, dtype=np.int64)])
    d = np.concatenate([dst.astype(np.int64), np.arange(N, dtype=np.int64)])
    order = np.argsort(d, kind="stable")
    s, d = s[order], d[order]

    # boundaries of every (core, tile) group in the dst-sorted edge list
    bounds = np.empty(NCORES * TILES + 1, np.int64)
    k = 0
    for c in range(NCORES):
        for t in range(TILES):
            lo = c * SHARD + min(t * P, SHARD)
            bounds[k] = np.searchsorted(d, lo, side="left")
            k += 1
    bounds[-1] = len(d)

    cnt = np.diff(bounds).reshape(NCORES, TILES)
    cpt = np.maximum((cnt + P - 1) // P, 1).max(axis=0)   # per-tile, cross-core
    choff = np.concatenate([[0], np.cumsum(cpt)]).astype(np.int64)
    totch = int(choff[-1])

    idx_all = np.zeros((NCORES, P, totch), np.int32)
    selt_all = np.zeros((NCORES, P, totch * P), np.float32)
    sel_all = np.zeros((NCORES, P, totch * P), NP_BF16)
    one = np.ones((), NP_BF16)
    for c in range(NCORES):
        for t in range(TILES):
            b0 = bounds[c * TILES + t]
            b1 = bounds[c * TILES + t + 1]
            n = b1 - b0
            if n == 0:
                continue
            es = s[b0:b1]
            jl = d[b0:b1] - (c * SHARD + t * P)
            off = choff[t]
            ch = np.arange(n) // P
            ep = np.arange(n) % P
            idx_all[c, ep, off + ch] = es
            selt_all[c, jl, (off + ch) * P + ep] = 1.0
            sel_all[c, ep, (off + ch) * P + jl] = one
    return cpt, choff, totch, idx_all, selt_all, sel_all


def _fold_weights(W, att_src, att_dst, Wsk):
    w_as = np.stack([W[:, h * C:(h + 1) * C] @ att_src[h] for h in range(H)], 1)
    w_ad = np.stack([W[:, h * C:(h + 1) * C] @ att_dst[h] for h in range(H)], 1)
    return np.concatenate([W, w_as, w_ad, Wsk], axis=1).astype(np.float32)


def _build_nc(cpt, choff, totch):
    nc = bass.Bass(
        "TRN2",
        num_devices=NCORES,
        use_seq_codegen=True,
        dynamic_dma_scratch_size=131072,
    )
    xs = nc.dram_tensor("xs", [NPAD, F_IN], F32, kind="ExternalInput")
    idx = nc.dram_tensor("idx", [P, totch], I32, kind="ExternalInput")
    selt = nc.dram_tensor("selt", [P, totch * P], F32, kind="ExternalInput")
    sel = nc.dram_tensor("sel", [P, totch * P], BF16, kind="ExternalInput")
    wall1 = nc.dram_tensor("wall1", [F_IN, PRJ], F32, kind="ExternalInput")
    wall2 = nc.dram_tensor("wall2", [HC, PRJ], F32, kind="ExternalInput")
    bb1 = nc.dram_tensor("bb1", [P, HC], F32, kind="ExternalInput")
    bb2 = nc.dram_tensor("bb2", [P, HC], F32, kind="ExternalInput")
    out = nc.dram_tensor("out", [SHARD, HC], F32, kind="ExternalOutput")

    layers = []
    for li in (1, 2):
        cc_in = nc.dram_tensor(f"cc_in{li}", [SHARD, ROW], F32, kind="Internal")
        table = nc.dram_tensor(
            f"table{li}", [N, ROW], F32, kind="Internal", addr_space="Shared"
        )
        ad = nc.dram_tensor(f"ad{li}", [NPAD, 2], F32, kind="Internal")
        skipb = nc.dram_tensor(f"skipb{li}", [NPAD, HC], F32, kind="Internal")
        layers.append((cc_in, table, ad, skipb))

    with tile.TileContext(nc) as tc:
        with (
            tc.tile_pool(name="const", bufs=1) as constp,
            tc.tile_pool(name="proj", bufs=6) as projp,
            tc.tile_pool(name="ppsum", bufs=2, space="PSUM") as ppsum,
            tc.tile_pool(name="gath", bufs=2) as gathp,
            tc.tile_pool(name="selp", bufs=2) as selp,
            tc.tile_pool(name="small", bufs=5) as smallp,
            tc.tile_pool(name="fwp", bufs=2) as fwp,
            tc.tile_pool(name="apsum", bufs=2, space="PSUM") as apsum,
            tc.tile_pool(name="finp", bufs=4) as finp,
        ):
            ident = constp.tile([P, P], F32)
            make_identity(nc, ident[:])
            walls = {}
            bbs = {}
            for li, wsrc, bsrc in ((1, wall1, bb1), (2, wall2, bb2)):
                wt = constp.tile([P, PRJ], F32, tag=f"wall{li}")
                nc.sync.dma_start(out=wt[:], in_=wsrc[:])
                bt = constp.tile([P, HC], F32, tag=f"bb{li}")
                nc.sync.dma_start(out=bt[:], in_=bsrc[:])
                walls[li] = wt
                bbs[li] = bt

            # zero pad tails of the a_d arrays (layer-2 writes only valid
            # rows; uninitialized DRAM could be NaN and would poison the
            # expansion matmul via 0*NaN)
            zt = constp.tile([P, HC], F32, tag="zero")
            nc.vector.memset(zt[:], 0.0)
            for li in (1, 2):
                nc.sync.dma_start(
                    out=layers[li - 1][2][SHARD:NPAD, :],
                    in_=zt[: NPAD - SHARD, :2],
                )

            # whole-layer gather index tile, loaded once
            it_all = constp.tile([P, totch], I32, tag="itall")
            nc.sync.dma_start(out=it_all[:], in_=idx[:])

            def proj_tile(li, t, xt):
                cc_in, table, ad, skipb = layers[li - 1]
                wt = walls[li]
                bt = bbs[li]
                if True:
                    rows = min(P, SHARD - t * P)
                    tp = ppsum.tile([P, F_IN], F32, tag="tp")
                    nc.tensor.transpose(out=tp[:], in_=xt[:], identity=ident[:])
                    xT = projp.tile([P, F_IN], F32, tag="xT")
                    nc.vector.tensor_copy(out=xT[:], in_=tp[:])
                    pj = ppsum.tile([P, PRJ], F32, tag="pj")
                    nc.tensor.matmul(
                        out=pj[:], lhsT=xT[:], rhs=wt[:], start=True, stop=True
                    )
                    rowst = projp.tile([P, ROW], F32, tag="rowst")
                    nc.vector.tensor_copy(out=rowst[:, 0:130], in_=pj[:, 0:130])
                    adt = projp.tile([P, 2], F32, tag="adt")
                    nc.scalar.copy(out=adt[:], in_=pj[:, 130:132])
                    skl = projp.tile([P, HC], F32, tag="skl")
                    nc.vector.tensor_add(
                        out=skl[:], in0=pj[:, 132:260], in1=bt[:]
                    )
                    nc.sync.dma_start(
                        out=cc_in[t * P : t * P + rows, :], in_=rowst[:rows, :]
                    )
                    nc.scalar.dma_start(
                        out=ad[t * P : t * P + rows, :], in_=adt[:rows, :]
                    )
                    nc.scalar.dma_start(
                        out=skipb[t * P : (t + 1) * P, :], in_=skl[:]
                    )

            def proj_collective(li):
                cc_in, table, ad, skipb = layers[li - 1]
                nc.gpsimd.collective_compute(
                    "AllGather",
                    mybir.AluOpType.bypass,
                    replica_groups=[list(range(NCORES))],
                    ins=[cc_in[:]],
                    outs=[table[:]],
                )

            def sweep(li, dst_dram, relu):
                cc_in, table, ad, skipb = layers[li - 1]
                for t in range(TILES):
                    rows = min(P, SHARD - t * P)
                    tt = int(cpt[t])
                    off = int(choff[t])
                    st = selp.tile([P, tt * P], F32, tag="selt")
                    nc.scalar.dma_start(
                        out=st[:], in_=selt[:, off * P : (off + tt) * P]
                    )
                    se = selp.tile([P, tt * P], BF16, tag="sel")
                    nc.sync.dma_start(
                        out=se[:], in_=sel[:, off * P : (off + tt) * P]
                    )
                    v2 = smallp.tile([P, 2], F32, tag="v2")
                    nc.scalar.dma_start(out=v2[:], in_=ad[t * P : (t + 1) * P, :])

                    gt = gathp.tile([P, tt, ROW], F32, tag="gt")
                    zp = apsum.tile([P, tt * 2], F32, tag="zp")
                    agg = apsum.tile([P, 130], F32, tag="agg")
                    for k in range(tt):
                        nc.gpsimd.indirect_dma_start(
                            out=gt[:, k, :],
                            out_offset=None,
                            in_=table[:],
                            in_offset=bass.IndirectOffsetOnAxis(
                                ap=it_all[:, off + k : off + k + 1], axis=0
                            ),
                        )
                        nc.tensor.matmul(
                            out=zp[:, 2 * k : 2 * k + 2],
                            lhsT=st[:, k * P : (k + 1) * P],
                            rhs=v2[:],
                            start=True,
                            stop=True,
                        )
                    z = smallp.tile([P, tt, 2], F32, tag="z")
                    nc.vector.tensor_add(
                        out=z[:],
                        in0=zp[:].rearrange("p (t two) -> p t two", two=2),
                        in1=gt[:, :, 128:130],
                    )
                    w1 = smallp.tile([P, tt, 2], F32, tag="w1")
                    nc.scalar.activation(
                        out=w1[:], in_=z[:], func=mybir.ActivationFunctionType.Exp
                    )
                    w2 = smallp.tile([P, tt, 2], F32, tag="w2")
                    nc.scalar.activation(
                        out=w2[:],
                        in_=z[:],
                        func=mybir.ActivationFunctionType.Exp,
                        scale=0.2,
                    )
                    w = smallp.tile([P, tt, 2], F32, tag="w")
                    nc.vector.tensor_tensor(
                        out=w[:], in0=w1[:], in1=w2[:], op=mybir.AluOpType.max
                    )
                    fw = fwp.tile([P, tt, 130], BF16, tag="fw")
                    nc.vector.tensor_copy(out=fw[:, :, 128:130], in_=w[:])
                    for k in range(tt):
                        for hh in range(H):
                            nc.vector.tensor_scalar_mul(
                                out=fw[:, k, hh * C : (hh + 1) * C],
                                in0=gt[:, k, hh * C : (hh + 1) * C],
                                scalar1=w[:, k, hh : hh + 1],
                            )
                        nc.tensor.matmul(
                            out=agg[:],
                            lhsT=se[:, k * P : (k + 1) * P],
                            rhs=fw[:, k, :],
                            start=(k == 0),
                            stop=(k == tt - 1),
                        )
                    rec = finp.tile([P, 2], F32, tag="rec")
                    nc.vector.reciprocal(out=rec[:], in_=agg[:, 128:130])
                    ot = finp.tile([P, HC], F32, tag="ot")
                    for hh in range(H):
                        nc.vector.tensor_scalar_mul(
                            out=ot[:, hh * C : (hh + 1) * C],
                            in0=agg[:, hh * C : (hh + 1) * C],
                            scalar1=rec[:, hh : hh + 1],
                        )
                    skl = finp.tile([P, HC], F32, tag="skl2")
                    nc.sync.dma_start(
                        out=skl[:], in_=skipb[t * P : (t + 1) * P, :]
                    )
                    ot2 = finp.tile([P, HC], F32, tag="ot2")
                    nc.vector.tensor_add(out=ot2[:], in0=ot[:], in1=skl[:])
                    if relu:
                        ot3 = finp.tile([P, HC], F32, tag="ot3")
                        nc.scalar.activation(
                            out=ot3[:],
                            in_=ot2[:],
                            func=mybir.ActivationFunctionType.Relu,
                        )
                    else:
                        ot3 = ot2
                    if dst_dram is not None:
                        nc.sync.dma_start(
                            out=dst_dram[t * P : t * P + rows, :],
                            in_=ot3[:rows, :],
                        )
                    else:
                        # layer-1: feed the tile straight into the layer-2
                        # projection (no HBM roundtrip)
                        proj_tile(2, t, ot3)

            def projection(li, src_dram):
                for t in range(TILES):
                    xt = projp.tile([P, F_IN], F32, tag="xt")
                    nc.scalar.dma_start(
                        out=xt[:], in_=src_dram[t * P : (t + 1) * P, :]
                    )
                    proj_tile(li, t, xt)

            projection(1, xs)
            proj_collective(1)
            sweep(1, None, relu=True)
            proj_collective(2)
            sweep(2, out, relu=False)

    _split_sync_waits(nc, limit=1)
    return nc


_CACHE = {}


def _get_program(src, dst):
    key = (hash(src.tobytes()), hash(dst.tobytes()))
    if key not in _CACHE:
        cpt, choff, totch, idx_all, selt_all, sel_all = _host_prep(src, dst)
        nc = _build_nc(cpt, choff, totch)
        _CACHE[key] = (nc, idx_all, selt_all, sel_all)
    return _CACHE[key]


def _run(inputs, trace=False):
    src = np.asarray(inputs["src"])
    dst = np.asarray(inputs["dst"])
    nc, idx_all, selt_all, sel_all = _get_program(src, dst)

    x = np.asarray(inputs["x"], np.float32)
    wall1 = _fold_weights(
        np.asarray(inputs["W1"]), np.asarray(inputs["att_src1"]),
        np.asarray(inputs["att_dst1"]), np.asarray(inputs["Wsk1"]),
    )
    wall2 = _fold_weights(
        np.asarray(inputs["W2"]), np.asarray(inputs["att_src2"]),
        np.asarray(inputs["att_dst2"]), np.asarray(inputs["Wsk2"]),
    )
    bb1 = np.tile(
        (np.asarray(inputs["b1"]) + np.asarray(inputs["bsk1"]))[None, :], (P, 1)
    ).astype(np.float32)
    bb2 = np.tile(
        (np.asarray(inputs["b2"]) + np.asarray(inputs["bsk2"]))[None, :], (P, 1)
    ).astype(np.float32)

    in_maps = []
    for c in range(NCORES):
        xsv = np.zeros((NPAD, F_IN), np.float32)
        xsv[:SHARD] = x[c * SHARD : (c + 1) * SHARD]
        in_maps.append(
            {
                "xs": xsv,
                "idx": idx_all[c],
                "selt": selt_all[c],
                "sel": sel_all[c],
                "wall1": wall1,
                "wall2": wall2,
                "bb1": bb1,
                "bb2": bb2,
            }
        )
    res = run_bass_kernel_spmd(
        nc, in_maps, core_ids=list(range(NCORES)), trace=trace
    )
    outp = np.concatenate([res.results[c]["out"] for c in range(NCORES)], axis=0)
    return outp.astype(np.float32), res.exec_time_ns


def kernel(**inputs) -> np.ndarray:
    out, _ = _run(inputs, trace=False)
    return out


def kernel_traced(**inputs):
    return _run(inputs, trace=True)



# revision 29
# speedup vs baseline: 1.1297x; 1.1297x over previous
"""Two-layer GAT (PyG GATConv semantics) on 8 Trainium2 NeuronCores.

v2 strategy (dst-sharded, batched SWDGE gathers, on-device masks):
- Destination nodes sharded contiguously across 8 cores (6250 each); edges
  (plus self-loops) sorted by dst, grouped per 128-dst tile, packed into
  128-edge chunks (chunk counts maxed across cores so the SPMD program is
  identical on every core).
- Layer-1 projection is REPLICATED: every core projects all N nodes from a
  host-pretransposed x (one matmul per 128-node tile) and writes a local
  bf16 table row [h0|1|h1|1|a_s|pad->256] (the 1-columns produce softmax
  denominators through the aggregation matmul). No collective for layer 1.
- Per tile, per-edge source rows arrive via dma_gather (batched SWDGE
  ucode: ~1us fixed + 0.34ns/row per instruction, vs ~1us PER 128-row
  chunk for generic indirect DMA — the baseline's bottleneck). dma_gather
  needs int16 indices, so each gather is split into a low half (table rows
  < 32768) and a high half; rows are 512B (256 bf16 cols) to satisfy the
  256B-multiple constraint — the pad bytes are free in DMA-engine time
  since sub-512B descriptors pay a 2x latency penalty anyway.
- Selection masks are generated on device from tiny per-chunk index
  vectors: Sel[ep,jl]=(iota==jlc) via one tensor_tensor with stride-0
  broadcast APs; SelT[jl,ep]=(lo<=iota<hi) via ge/lt/mult. No mask
  streaming from HBM (the baseline's other big cost).
- Per-edge logits z = a_s[src] (gathered) + a_d[dst] (SelT matmul); edge
  weight w = max(exp(z), exp(0.2 z)) (exp(leaky_relu) factorization;
  softmax scale-invariance makes max subtraction unnecessary). Gathered
  rows are scaled in place by w per head, then ONE PE matmul per chunk
  accumulates numerators and denominators in PSUM.
- Layer-2 projection is fused into sweep-1 per tile; its AllGather is
  split into 5 pieces issued as tile groups complete, overlapping the
  collective under sweep-1 compute. Gather indices for sweep-2 are
  host-remapped into the piece-major AllGather output layout.
"""

import sys

if "/opt/trn_rl_repo" not in sys.path:
    sys.path.insert(0, "/opt/trn_rl_repo")

import numpy as np

import bass_rust
import concourse.bass as bass
import concourse.mybir as mybir
import concourse.tile as tile
from concourse import library_config
from concourse.bass_utils import run_bass_kernel_spmd

N, E, F_IN, H, C = 50000, 800000, 128, 2, 64
HC = H * C
NCORES = 8
SHARD = N // NCORES            # 6250
P = 128
TILES = (SHARD + P - 1) // P   # 49
NPAD = TILES * P               # 6272
NT1 = (N + P - 1) // P         # 391 global tiles
NPAD1 = NT1 * P                # 50048
ROW = 256                      # gather row: h0(64)|1|h1(64)|1|a_s(2)|junk
CC = 132                       # meaningful row columns
PRJ = 262                      # wall cols: Wh0 |0| Wh1 |0| w_as | w_ad | Wsk
HALF = 32768                   # int16 index limit for dma_gather
GMAX = 8                       # max 128-row chunks per dma_gather instruction
SLAB = 8                       # proj-1 tiles per DMA slab
NGRP = 5                       # AllGather pieces for layer 2
GRP_TILES = [10, 10, 10, 10, 9]
GRP_ROWS = [t * P for t in GRP_TILES]
GRP_T0 = [0, 10, 20, 30, 40]
GRP_R0 = [0, 1280, 2560, 3840, 5120]

F32 = mybir.dt.float32
BF16 = mybir.dt.bfloat16
I16 = mybir.dt.int16
NP_BF16 = mybir.dt.np(BF16)


def _split_sync_waits(nc, limit=1):
    """walrus in this container rejects >1 sync wait per instruction; move
    excess waits onto NoOps inserted just before the offending one."""
    ctr = [0]

    def fresh_noop(engine, waits):
        ctr[0] += 1
        return mybir.InstNoOp(
            name=f"waitsplit-{ctr[0]}",
            engine=engine,
            bass_nofuse=True,
            sync_info=mybir.SyncInfo(on_wait=list(waits), on_update=[]),
        )

    for f in nc.m.functions:
        for bb in f.blocks:
            out = []
            changed = False
            for ins in bb.instructions:
                si = ins.sync_info
                waits = list(si.on_wait) if si else []
                if len(waits) > limit:
                    changed = True
                    excess, keep = waits[:-limit], waits[-limit:]
                    for i in range(0, len(excess), limit):
                        noop = fresh_noop(ins.engine, excess[i : i + limit])
                        nc.register_instruction(noop, overwrite=True)
                        out.append(noop)
                    ins.sync_info = mybir.SyncInfo(
                        on_wait=keep, on_update=list(si.on_update)
                    )
                out.append(ins)
            if changed:
                bb.instructions = out
    return ctr[0]


def _bc_last(ap2, n):
    """Append a stride-0 broadcast as a new last AP dim."""
    a = [list(d) for d in ap2.ap]
    a.append([0, n])
    return bass.AP(ap2.tensor, ap2.offset, a)


def _bc_rep1(ap3, n):
    """Replace a trailing size-1 AP dim with a stride-0 broadcast of size n."""
    a = [list(d) for d in ap3.ap]
    assert a[-1][1] == 1, a
    a[-1] = [0, n]
    return bass.AP(ap3.tensor, ap3.offset, a)


def _wrap16(idx_i64):
    """Pack an index list into the dma_gather layout: [128, n/16] int16 with
    idx[i] at [i % 16, i // 16], replicated into all 8 16-partition blocks."""
    n = len(idx_i64)
    assert n % 16 == 0
    t = np.zeros((16, n // 16), np.int16)
    t[np.arange(n) % 16, np.arange(n) // 16] = idx_i64.astype(np.int16)
    return np.tile(t, (8, 1))


class _LayerPlan:
    """Per-layer edge chunking: per tile, lo-half chunks then hi-half chunks
    (split by table row < HALF), each half dst-sorted so jl runs stay
    contiguous per chunk."""

    def __init__(self):
        self.cptl = np.zeros(TILES, np.int64)
        self.cpth = np.zeros(TILES, np.int64)
        self.offl = None
        self.offh = None
        self.choff = None
        self.totch = 0
        self.idxl = None    # [NCORES][tile] -> list of int idx (lo, rebased)
        self.idxh = None
        self.meta = None    # [NCORES, P, 3, totch]


def _host_prep(src, dst):
    s = np.concatenate([src.astype(np.int64), np.arange(N, dtype=np.int64)])
    d = np.concatenate([dst.astype(np.int64), np.arange(N, dtype=np.int64)])
    order = np.argsort(d, kind="stable")
    s, d = s[order], d[order]

    bounds = np.empty(NCORES * TILES + 1, np.int64)
    k = 0
    for c in range(NCORES):
        for t in range(TILES):
            lo = c * SHARD + min(t * P, SHARD)
            bounds[k] = np.searchsorted(d, lo, side="left")
            k += 1
    bounds[-1] = len(d)

    # remap node id -> row in the piece-major layer-2 AllGather output
    nid = np.arange(N, dtype=np.int64)
    cc_ = nid // SHARD
    rr = nid - cc_ * SHARD
    gg = np.minimum(rr // 1280, NGRP - 1)
    g_rows = np.asarray(GRP_ROWS, np.int64)[gg]
    g_r0 = np.asarray(GRP_R0, np.int64)[gg]
    remap = 8 * g_r0 + cc_ * g_rows + (rr - g_r0)

    plans = [_LayerPlan(), _LayerPlan()]
    rows_of = [lambda es: es, lambda es: remap[es]]

    # first pass: chunk counts per (tile, half), maxed across cores
    per_ct = {}
    for c in range(NCORES):
        for t in range(TILES):
            b0, b1 = bounds[c * TILES + t], bounds[c * TILES + t + 1]
            es = s[b0:b1]
            jl = d[b0:b1] - (c * SHARD + t * P)
            per_ct[(c, t)] = (es, jl)
            for li in range(2):
                rows = rows_of[li](es)
                nlo = int((rows < HALF).sum())
                nhi = len(rows) - nlo
                plans[li].cptl[t] = max(plans[li].cptl[t], (nlo + P - 1) // P)
                plans[li].cpth[t] = max(plans[li].cpth[t], (nhi + P - 1) // P)

    jgrid = np.arange(P)
    for li in range(2):
        pl = plans[li]
        pl.cptl = np.maximum(pl.cptl, 1)
        cpt = pl.cptl + pl.cpth
        pl.choff = np.concatenate([[0], np.cumsum(cpt)]).astype(np.int64)
        pl.offl = np.concatenate([[0], np.cumsum(pl.cptl)]).astype(np.int64)
        pl.offh = np.concatenate([[0], np.cumsum(pl.cpth)]).astype(np.int64)
        pl.totch = int(pl.choff[-1])
        pl.meta = np.zeros((NCORES, P, 3, pl.totch), np.float32)
        pl.meta[:, :, 0, :] = -1.0
        pl.idxl = np.zeros((NCORES, 128, int(pl.offl[-1]) * 8), np.int16)
        pl.idxh = np.zeros((NCORES, 128, max(int(pl.offh[-1]), 1) * 8), np.int16)
        for c in range(NCORES):
            for t in range(TILES):
                es, jl = per_ct[(c, t)]
                rows = rows_of[li](es)
                lom = rows < HALF
                for half, (hes, hjl) in enumerate(
                    ((rows[lom], jl[lom]), (rows[~lom] - HALF, jl[~lom]))
                ):
                    nch = int(pl.cptl[t] if half == 0 else pl.cpth[t])
                    if nch == 0:
                        continue
                    npad = nch * P
                    ii = np.zeros(npad, np.int64)
                    ii[: len(hes)] = hes
                    wrapped = _wrap16(ii)
                    o8 = int(pl.offl[t] if half == 0 else pl.offh[t]) * 8
                    dst_arr = pl.idxl if half == 0 else pl.idxh
                    dst_arr[c][:, o8 : o8 + nch * 8] = wrapped
                    kbase = int(
                        pl.choff[t] + (0 if half == 0 else pl.cptl[t])
                    )
                    n = len(hjl)
                    for ch in range(nch):
                        e0, e1 = ch * P, min((ch + 1) * P, n)
                        if e0 >= n:
                            pl.meta[c, :, 1, kbase + ch] = 0
                            pl.meta[c, :, 2, kbase + ch] = 0
                            continue
                        jc = hjl[e0:e1]
                        m = e1 - e0
                        pl.meta[c, :m, 0, kbase + ch] = jc
                        pl.meta[c, :, 1, kbase + ch] = np.searchsorted(
                            jc, jgrid, side="left"
                        )
                        pl.meta[c, :, 2, kbase + ch] = np.searchsorted(
                            jc, jgrid, side="right"
                        )
    return plans


def _fold_wall(W, att_src, att_dst, Wsk):
    w_as = np.stack([W[:, h * C:(h + 1) * C] @ att_src[h] for h in range(H)], 1)
    w_ad = np.stack([W[:, h * C:(h + 1) * C] @ att_dst[h] for h in range(H)], 1)
    z1 = np.zeros((F_IN, 1), np.float32)
    return np.concatenate(
        [W[:, 0:C], z1, W[:, C:HC], z1, w_as, w_ad, Wsk], axis=1
    ).astype(NP_BF16)


def _build_nc(plans):
    maxtt = int(max((plans[0].cptl + plans[0].cpth).max(),
                    (plans[1].cptl + plans[1].cpth).max()))
    nc = bass.Bass(
        "TRN2",
        num_devices=NCORES,
        use_seq_codegen=True,
        dynamic_dma_scratch_size=49152,
    )
    xt1 = nc.dram_tensor("xt1", [P, NPAD1], BF16, kind="ExternalInput")
    xto = nc.dram_tensor("xto", [P, NPAD], BF16, kind="ExternalInput")
    identd = nc.dram_tensor("identd", [P, P], BF16, kind="ExternalInput")
    iotad = nc.dram_tensor("iotad", [P, P], F32, kind="ExternalInput")
    wall1 = nc.dram_tensor("wall1", [P, PRJ], BF16, kind="ExternalInput")
    wall2 = nc.dram_tensor("wall2", [P, PRJ], BF16, kind="ExternalInput")
    bb1 = nc.dram_tensor("bb1", [P, HC], F32, kind="ExternalInput")
    bb2 = nc.dram_tensor("bb2", [P, HC], F32, kind="ExternalInput")
    metad = [
        nc.dram_tensor(f"meta{li}", [P, 3 * plans[li].totch], F32,
                       kind="ExternalInput")
        for li in range(2)
    ]
    idxld = [
        nc.dram_tensor(f"idxl{li}", [P, plans[li].idxl.shape[-1]], I16,
                       kind="ExternalInput")
        for li in range(2)
    ]
    idxhd = [
        nc.dram_tensor(f"idxh{li}", [P, plans[li].idxh.shape[-1]], I16,
                       kind="ExternalInput")
        for li in range(2)
    ]
    table1 = nc.dram_tensor("table1", [NPAD1, ROW], BF16, kind="Internal")
    cc2g = [
        nc.dram_tensor(f"cc2g{g}", [GRP_ROWS[g], ROW], BF16, kind="Internal")
        for g in range(NGRP)
    ]
    table2 = nc.dram_tensor(
        "table2", [8 * NPAD, ROW], BF16, kind="Internal", addr_space="Shared"
    )
    out = nc.dram_tensor("out", [SHARD, HC], F32, kind="ExternalOutput")

    with tile.TileContext(nc) as tc:
        with (
            tc.tile_pool(name="const", bufs=1) as constp,
            tc.tile_pool(name="xsl", bufs=2) as xsl,
            tc.tile_pool(name="rsl", bufs=2) as rsl,
            tc.tile_pool(name="ppj", bufs=2, space="PSUM") as ppj,
            tc.tile_pool(name="gth", bufs=2) as gth,
            tc.tile_pool(name="idxp", bufs=2) as idxp,
            tc.tile_pool(name="metap", bufs=2) as metap,
            tc.tile_pool(name="msk", bufs=2) as msk,
            tc.tile_pool(name="zw", bufs=2) as zw,
            tc.tile_pool(name="zps", bufs=2, space="PSUM") as zps,
            tc.tile_pool(name="aps", bufs=2, space="PSUM") as aps,
            tc.tile_pool(name="tps", bufs=2, space="PSUM") as tps,
            tc.tile_pool(name="fin", bufs=2) as fin,
            tc.tile_pool(name="keep", bufs=1) as keep,
        ):
            reload_inst = nc.gpsimd.load_library(library_config.mlp)
            ident = constp.tile([P, P], BF16)
            nc.sync.dma_start(out=ident[:], in_=identd[:])
            w1t = constp.tile([P, PRJ], BF16, tag="wall1")
            nc.sync.dma_start(out=w1t[:], in_=wall1[:])
            w2t = constp.tile([P, PRJ], BF16, tag="wall2")
            nc.sync.dma_start(out=w2t[:], in_=wall2[:])
            bb1t = constp.tile([P, HC], F32, tag="bb1")
            nc.sync.dma_start(out=bb1t[:], in_=bb1[:])
            bb2t = constp.tile([P, HC], F32, tag="bb2")
            nc.sync.dma_start(out=bb2t[:], in_=bb2[:])
            xot = constp.tile([P, NPAD], BF16, tag="xot")
            nc.sync.dma_start(out=xot[:], in_=xto[:])
            iotf = constp.tile([P, P], F32, tag="iotf")
            nc.sync.dma_start(out=iotf[:], in_=iotad[:])

            v2_tiles = {}
            skl_tiles = {}
            nreg_cache = {}

            def nreg(n):
                if n not in nreg_cache:
                    nreg_cache[n] = nc.gpsimd.to_reg(n)
                return nreg_cache[n]

            def rep_iota(tt):
                a = iotf[:]
                return bass.AP(
                    a.tensor, a.offset, [list(a.ap[0]), [0, tt], list(a.ap[1])]
                )

            # ---------- replicated layer-1 projection ----------
            for s0 in range(0, NT1, SLAB):
                nt = min(SLAB, NT1 - s0)
                xs = xsl.tile([P, nt * P], BF16, tag=f"xs{nt}")
                nc.sync.dma_start(
                    out=xs[:], in_=xt1[:, s0 * P : (s0 + nt) * P]
                )
                rows_t = rsl.tile([P, nt, CC], BF16, tag=f"rw{nt}")
                for k in range(nt):
                    pj = ppj.tile([P, PRJ], F32, tag="pj")
                    nc.tensor.matmul(
                        out=pj[:, 0:CC],
                        lhsT=xs[:, k * P : (k + 1) * P],
                        rhs=w1t[:, 0:CC],
                        start=True,
                        stop=True,
                    )
                    nc.vector.tensor_copy(out=rows_t[:, k, :], in_=pj[:, 0:CC])
                ra = rows_t[:]
                ones_ap = bass.AP(
                    ra.tensor,
                    ra.offset + 64,
                    [list(ra.ap[0]), [CC, nt], [65, 2]],
                )
                nc.vector.memset(ones_ap, 1.0)
                nc.sync.dma_start(
                    out=table1[s0 * P : (s0 + nt) * P, 0:CC].rearrange(
                        "(k p) c -> p k c", p=P
                    ),
                    in_=rows_t[:],
                )

            # ---------- own-shard layer-1 a_d / skip projection ----------
            for t in range(TILES):
                pjo = ppj.tile([P, PRJ], F32, tag="pj")
                nc.tensor.matmul(
                    out=pjo[:, 0 : PRJ - CC],
                    lhsT=xot[:, t * P : (t + 1) * P],
                    rhs=w1t[:, CC:PRJ],
                    start=True,
                    stop=True,
                )
                v2t = keep.tile([P, 2], BF16, tag=f"v1_{t}")
                nc.scalar.copy(out=v2t[:], in_=pjo[:, 0:2])
                sklt = keep.tile([P, HC], BF16, tag=f"s1_{t}")
                nc.vector.tensor_add(out=sklt[:], in0=pjo[:, 2:130], in1=bb1t[:])
                v2_tiles[(1, t)] = v2t
                skl_tiles[(1, t)] = sklt

            # ---------- sweeps ----------
            def sweep(li):
                pl = plans[li - 1]
                table = table1 if li == 1 else table2
                nrow = NPAD1 if li == 1 else 8 * NPAD
                for t in range(TILES):
                    ttl = int(pl.cptl[t])
                    tth = int(pl.cpth[t])
                    tt = ttl + tth
                    off = int(pl.choff[t])
                    rows = min(P, SHARD - t * P)
                    v2t = v2_tiles[(li, t)]
                    sklt = skl_tiles[(li, t)]

                    gt = gth.tile([P, maxtt, ROW], BF16, tag="gt")
                    itl = idxp.tile([P, maxtt * 8], I16, tag="itl")
                    nc.sync.dma_start(
                        out=itl[:, : ttl * 8],
                        in_=idxld[li - 1][
                            :, int(pl.offl[t]) * 8 : (int(pl.offl[t]) + ttl) * 8
                        ],
                    )
                    for p0 in range(0, ttl, GMAX):
                        pn = min(GMAX, ttl - p0)
                        g1 = nc.gpsimd.dma_gather(
                            gt[:, p0 : p0 + pn, :],
                            table[0:HALF, :],
                            itl[:, p0 * 8 : (p0 + pn) * 8],
                            pn * P,
                            nreg(pn * P),
                            ROW,
                        )
                        bass_rust.add_dep_helper(
                            g1.ins, reload_inst.ins, sync=False, reason="lib"
                        )
                    if tth > 0:
                        ith = idxp.tile([P, maxtt * 8], I16, tag="ith")
                        nc.sync.dma_start(
                            out=ith[:, : tth * 8],
                            in_=idxhd[li - 1][
                                :,
                                int(pl.offh[t]) * 8 : (int(pl.offh[t]) + tth) * 8,
                            ],
                        )
                        for p0 in range(0, tth, GMAX):
                            pn = min(GMAX, tth - p0)
                            g2 = nc.gpsimd.dma_gather(
                                gt[:, ttl + p0 : ttl + p0 + pn, :],
                                table[HALF:nrow, :],
                                ith[:, p0 * 8 : (p0 + pn) * 8],
                                pn * P,
                                nreg(pn * P),
                                ROW,
                            )
                            bass_rust.add_dep_helper(
                                g2.ins, reload_inst.ins, sync=False, reason="lib"
                            )

                    mt = metap.tile([P, 3, maxtt], F32, tag="mt")
                    nc.scalar.dma_start(
                        out=mt[:, :, :tt],
                        in_=metad[li - 1][:]
                        .rearrange("p (k t) -> p k t", k=3)[:, :, off : off + tt],
                    )

                    # SelT[jl, k, ep] = (lo <= ep < hi)
                    sta = msk.tile([P, maxtt, P], BF16, tag="sta")
                    stb = msk.tile([P, maxtt, P], BF16, tag="stb")
                    nc.vector.tensor_tensor(
                        out=sta[:, :tt, :],
                        in0=rep_iota(tt),
                        in1=_bc_last(mt[:, 1, :tt], P),
                        op=mybir.AluOpType.is_ge,
                    )
                    nc.vector.tensor_tensor(
                        out=stb[:, :tt, :],
                        in0=rep_iota(tt),
                        in1=_bc_last(mt[:, 2, :tt], P),
                        op=mybir.AluOpType.is_lt,
                    )
                    nc.vector.tensor_tensor(
                        out=sta[:, :tt, :],
                        in0=sta[:, :tt, :],
                        in1=stb[:, :tt, :],
                        op=mybir.AluOpType.mult,
                    )
                    # zp[ep, k, h] = a_d[jl(ep), h]
                    zpp = zps.tile([P, maxtt, 2], F32, tag="zp")
                    for k in range(tt):
                        nc.tensor.matmul(
                            out=zpp[:, k, :],
                            lhsT=sta[:, k, :],
                            rhs=v2t[:],
                            start=True,
                            stop=True,
                        )
                    asf = zw.tile([P, maxtt, 2], F32, tag="asf")
                    nc.vector.tensor_copy(
                        out=asf[:, :tt, :], in_=gt[:, :tt, 130:132]
                    )
                    zt = zw.tile([P, maxtt, 2], F32, tag="z")
                    nc.vector.tensor_add(
                        out=zt[:, :tt, :], in0=zpp[:, :tt, :], in1=asf[:, :tt, :]
                    )
                    wa = zw.tile([P, maxtt, 2], BF16, tag="wa")
                    nc.scalar.activation(
                        out=wa[:, :tt, :],
                        in_=zt[:, :tt, :],
                        func=mybir.ActivationFunctionType.Exp,
                    )
                    wb = zw.tile([P, maxtt, 2], BF16, tag="wb")
                    nc.scalar.activation(
                        out=wb[:, :tt, :],
                        in_=zt[:, :tt, :],
                        func=mybir.ActivationFunctionType.Exp,
                        scale=0.2,
                    )
                    wt = zw.tile([P, maxtt, 2], BF16, tag="w")
                    nc.vector.tensor_tensor(
                        out=wt[:, :tt, :],
                        in0=wa[:, :tt, :],
                        in1=wb[:, :tt, :],
                        op=mybir.AluOpType.max,
                    )
                    # Sel[ep, k, jl] = (jl == jlc[ep, k])
                    sel = msk.tile([P, maxtt, P], BF16, tag="sel")
                    nc.vector.tensor_tensor(
                        out=sel[:, :tt, :],
                        in0=rep_iota(tt),
                        in1=_bc_last(mt[:, 0, :tt], P),
                        op=mybir.AluOpType.is_equal,
                    )
                    # in-place scale of gathered rows by per-head weight
                    for hh in range(H):
                        c0 = hh * 65
                        nc.vector.tensor_tensor(
                            out=gt[:, :tt, c0 : c0 + 65],
                            in0=gt[:, :tt, c0 : c0 + 65],
                            in1=_bc_rep1(wt[:, :tt, hh : hh + 1], 65),
                            op=mybir.AluOpType.mult,
                        )
                    psa = aps.tile([P, 130], F32, tag="agg")
                    for k in range(tt):
                        nc.tensor.matmul(
                            out=psa[:],
                            lhsT=sel[:, k, :],
                            rhs=gt[:, k, 0:130],
                            start=(k == 0),
                            stop=(k == tt - 1),
                        )
                    rec = fin.tile([P, 2], F32, tag="rec")
                    pa = psa[:]
                    nc.vector.reciprocal(
                        out=rec[:],
                        in_=bass.AP(
                            pa.tensor, pa.offset + 64, [list(pa.ap[0]), [65, 2]]
                        ),
                    )
                    # pad dst rows have denominator 0 -> rec=inf, and the
                    # 0*inf=NaN would poison downstream matmuls via the
                    # fused layer-2 projection; clamp so 0*BIG=0 instead
                    nc.vector.tensor_scalar_min(
                        out=rec[:], in0=rec[:], scalar1=1e30
                    )
                    ot = fin.tile([P, HC], F32, tag="ot")
                    for hh in range(H):
                        nc.vector.tensor_scalar_mul(
                            out=ot[:, hh * C : (hh + 1) * C],
                            in0=psa[:, hh * 65 : hh * 65 + 64],
                            scalar1=rec[:, hh : hh + 1],
                        )
                    sm = fin.tile([P, HC], F32, tag="sm")
                    nc.vector.tensor_add(out=sm[:], in0=ot[:], in1=sklt[:])

                    if li == 1:
                        h1t = fin.tile([P, HC], BF16, tag="h1")
                        nc.scalar.activation(
                            out=h1t[:],
                            in_=sm[:],
                            func=mybir.ActivationFunctionType.Relu,
                        )
                        tp = tps.tile([P, P], BF16, tag="tp")
                        nc.tensor.transpose(
                            out=tp[:], in_=h1t[:], identity=ident[:]
                        )
                        xT2 = fin.tile([P, P], BF16, tag="xT2")
                        nc.scalar.copy(out=xT2[:], in_=tp[:])
                        pj2 = ppj.tile([P, PRJ], F32, tag="pj")
                        nc.tensor.matmul(
                            out=pj2[:],
                            lhsT=xT2[:],
                            rhs=w2t[:],
                            start=True,
                            stop=True,
                        )
                        rowt = fin.tile([P, CC], BF16, tag="row2")
                        nc.vector.tensor_copy(out=rowt[:], in_=pj2[:, 0:CC])
                        rp = rowt[:]
                        nc.vector.memset(
                            bass.AP(
                                rp.tensor,
                                rp.offset + 64,
                                [list(rp.ap[0]), [65, 2]],
                            ),
                            1.0,
                        )
                        v22 = keep.tile([P, 2], BF16, tag=f"v2_{t}")
                        nc.scalar.copy(out=v22[:], in_=pj2[:, CC : CC + 2])
                        skl2 = keep.tile([P, HC], BF16, tag=f"s2_{t}")
                        nc.vector.tensor_add(
                            out=skl2[:], in0=pj2[:, CC + 2 : PRJ], in1=bb2t[:]
                        )
                        v2_tiles[(2, t)] = v22
                        skl_tiles[(2, t)] = skl2
                        g = min(t // 10, NGRP - 1)
                        nc.scalar.dma_start(
                            out=cc2g[g][
                                (t - GRP_T0[g]) * P : (t - GRP_T0[g] + 1) * P,
                                0:CC,
                            ],
                            in_=rowt[:],
                        )
                        if t - GRP_T0[g] == GRP_TILES[g] - 1:
                            nc.gpsimd.collective_compute(
                                "AllGather",
                                mybir.AluOpType.bypass,
                                replica_groups=[list(range(NCORES))],
                                ins=[cc2g[g][:]],
                                outs=[
                                    table2[
                                        8 * GRP_R0[g] : 8 * GRP_R0[g]
                                        + 8 * GRP_ROWS[g],
                                        :,
                                    ]
                                ],
                            )
                    else:
                        nc.scalar.dma_start(
                            out=out[t * P : t * P + rows, :], in_=sm[:rows, :]
                        )

            sweep(1)
            sweep(2)

    _split_sync_waits(nc, limit=1)
    mybir.codegen_inst_isa_subclasses(nc)
    return nc


_CACHE = {}


def _get_program(src, dst):
    key = (hash(src.tobytes()), hash(dst.tobytes()))
    if key not in _CACHE:
        plans = _host_prep(src, dst)
        nc = _build_nc(plans)
        _CACHE[key] = (nc, plans)
    return _CACHE[key]


def _run(inputs, trace=False):
    src = np.asarray(inputs["src"])
    dst = np.asarray(inputs["dst"])
    nc, plans = _get_program(src, dst)

    x = np.asarray(inputs["x"], np.float32)
    wall1 = _fold_wall(
        np.asarray(inputs["W1"]), np.asarray(inputs["att_src1"]),
        np.asarray(inputs["att_dst1"]), np.asarray(inputs["Wsk1"]),
    )
    wall2 = _fold_wall(
        np.asarray(inputs["W2"]), np.asarray(inputs["att_src2"]),
        np.asarray(inputs["att_dst2"]), np.asarray(inputs["Wsk2"]),
    )
    bb1 = np.tile(
        (np.asarray(inputs["b1"]) + np.asarray(inputs["bsk1"]))[None, :], (P, 1)
    ).astype(np.float32)
    bb2 = np.tile(
        (np.asarray(inputs["b2"]) + np.asarray(inputs["bsk2"]))[None, :], (P, 1)
    ).astype(np.float32)

    xp = np.zeros((NPAD1, F_IN), np.float32)
    xp[:N] = x
    xt1 = np.ascontiguousarray(xp.T).astype(NP_BF16)

    in_maps = []
    for c in range(NCORES):
        xop = np.zeros((NPAD, F_IN), np.float32)
        xop[:SHARD] = x[c * SHARD : (c + 1) * SHARD]
        im = {
            "xt1": xt1,
            "xto": np.ascontiguousarray(xop.T).astype(NP_BF16),
            "identd": np.eye(P, dtype=NP_BF16),
            "iotad": np.tile(
                np.arange(P, dtype=np.float32)[None, :], (P, 1)
            ),
            "wall1": wall1,
            "wall2": wall2,
            "bb1": bb1,
            "bb2": bb2,
        }
        for li in range(2):
            pl = plans[li]
            im[f"meta{li}"] = pl.meta[c].reshape(P, 3 * pl.totch)
            im[f"idxl{li}"] = pl.idxl[c]
            im[f"idxh{li}"] = pl.idxh[c]
        in_maps.append(im)
    res = run_bass_kernel_spmd(
        nc, in_maps, core_ids=list(range(NCORES)), trace=trace
    )
    outp = np.concatenate([res.results[c]["out"] for c in range(NCORES)], axis=0)
    return outp.astype(np.float32), res.exec_time_ns


def kernel(**inputs) -> np.ndarray:
    out, _ = _run(inputs, trace=False)
    return out


def kernel_traced(**inputs):
    return _run(inputs, trace=True)
